# revision 35
# baseline (speedup 1.0000x reference)
"""Trainium2 Bass kernel for nn_DomainBlock_1520418423078 (GNN message passing).

out[e] = (x[src]+x[dst]) @ w_x + ew[e] @ w_ew_i + (sum_ew[src]+sum_ew[dst]) @ w_ew_j
       = y[src[e]] + y[dst[e]] + ew[e] @ w_ew_i,
  where sum_ew = segment_sum(ew, src),  y = x @ w_x + sum_ew @ w_ew_j.

Two SPMD launches on 8 NeuronCores (edges sharded by src range), all large
streams bf16 (ew stream fp8e4): tolerance budget is 2e-2, we sit at ~9e-3.

  launch 1: per-core segment_sum via slot-padded sorted stream: DVE
            tree-add within 8-slot blocks; band-packed bins (<=26 nodes,
            <=128 blocks) so the DVE one-hot build is 26 cols/tile; PE
            scatter matmul per bin into disjoint psum column bands (3
            windows share a psum bank via partition thirds); then
            y = [x;sum_ew] @ [w_x;w_ew_j].  One bulk y writeback.
  host:     index y rows into a per-edge ysum = y[src]+y[dst] stream and
            pack streams into PE-friendly transposed layouts (data staging).
  launch 2: stream ew (pre-transposed 4-edge-packed, fp8) and ysum (bf16);
            PE computes ew @ w_ew_i via one block-diagonal matmul per 512
            edges (no on-device transposes); DVE adds psum + ysum -> bf16.

Hard-won scheduling facts: Pool's bf16 add is ~5x slower than nominal and
it can't read PSUM or run is_equal, so all elementwise goes to DVE; engines
that issue DMAs (SP/Act/Pool-SWDGE) must not also run compute that blocks
their sequencer, or the DMA stream stalls (cost ~40us/launch).
"""

import math
import os

import numpy as np
import ml_dtypes

os.environ.setdefault("NEURON_RT_RESET_CORES", "1")

import concourse.bacc as bacc
import concourse.bass as bass
import concourse.mybir as mybir
import concourse.tile as tile
from concourse import bass_utils

N_CORES = 8
N_NODES = 50000
X_DIM = 32
NODES_PER_CORE = N_NODES // N_CORES          # 6250
N_WIN = 49                                   # 128-node windows per core
TILES_PER_WIN = 5                            # bands (128-block tiles) per window
WIN_GROUPS = 7                               # windows processed per batch
N_WG = N_WIN // WIN_GROUPS                   # 7 window groups
NODE_SLOTS = N_WIN * 128                     # 6272 table rows per core
N_L1_TILES = N_WIN * TILES_PER_WIN           # 245
SLOTS_PER_CORE = N_L1_TILES * 1024           # 250880 slot rows
BAND_W = [26, 26, 26, 26, 24]                # node slots per band (sum 128)
BAND_C0 = [0, 26, 52, 78, 104]               # band start slot
L1_BATCH = int(os.environ.get("L1_BATCH", "15"))
L1_MODE = os.environ.get("L1_MODE", "full")  # full | dmaonly
L1_S1 = os.environ.get("L1_S1", "dve")       # mix | dve | pool
                                             # (Pool bf16 add is ~5x slower
                                             # than the cost model claims)
L2_MODE = os.environ.get("L2_MODE", "full")  # full | dmaonly
L2_ADD = os.environ.get("L2_ADD", "dve")     # mix | dve  (mix stalls the
                                             # act DMA-issue queue)
EDGE_BATCH = int(os.environ.get("EDGE_BATCH", "8192"))
L2_BUFS = int(os.environ.get("L2_BUFS", "3"))
L1_QS = os.environ.get("L1_QS", "sp,act,act").split(",")
L1_SPLIT = int(os.environ.get("L1_SPLIT", "1"))  # sub-DMAs per slot batch
L1_YTAIL = os.environ.get("L1_YTAIL", "group")   # group | end
L2_QS = os.environ.get("L2_QS", "sp,act,pool").split(",")


def _qeng(nc, name):
    return {"sp": nc.sync, "act": nc.scalar, "dve": nc.vector,
            "pool": nc.gpsimd}[name]
F32 = mybir.dt.float32
BF = mybir.dt.bfloat16
NPBF = ml_dtypes.bfloat16
L2_EW = os.environ.get("L2_EW", "fp8")       # fp8 | bf16  (ew4T/W4 dtype)
EW_DT = mybir.dt.float8e4 if L2_EW == "fp8" else BF
NPEW = mybir.dt.np(EW_DT)

_programs = {}


# ---------------------------------------------------------------- launch 1

def _build_launch1(reps=1):
    nc = bacc.Bacc("TRN2", target_bir_lowering=False, debug=False,
                   enable_asserts=False, num_devices=N_CORES)
    d_slots = nc.dram_tensor("slots", [128, N_L1_TILES * 256], BF,
                             kind="ExternalInput")
    d_blkT = nc.dram_tensor("blkT", [128, N_L1_TILES], BF,
                            kind="ExternalInput")
    d_xT = nc.dram_tensor("xT", [32, NODE_SLOTS], BF, kind="ExternalInput")
    d_iota = nc.dram_tensor("iota", [128, 128], BF, kind="ExternalInput")
    d_wcat = nc.dram_tensor("wcat", [64, 32], BF, kind="ExternalInput")
    d_y = nc.dram_tensor("y", [NODE_SLOTS, 32], BF, kind="ExternalOutput")

    with tile.TileContext(nc) as tc:
        with tc.tile_pool(name="const", bufs=1) as const, \
             tc.tile_pool(name="sbuf", bufs=3) as sbuf, \
             tc.tile_pool(name="psum", bufs=4, space="PSUM") as psum:
            iota_t = const.tile([128, 128], BF)
            nc.sync.dma_start(iota_t[:], d_iota[:])
            wcat_t = const.tile([64, 32], BF)
            nc.sync.dma_start(wcat_t[:], d_wcat[:])
            blkT_t = const.tile([128, N_L1_TILES], BF)
            nc.sync.dma_start(blkT_t[:], d_blkT[:])
            # stacked: rows 0-31 xT, rows 32-63 sum_ewT (window flushes)
            stacked = const.tile([64, NODE_SLOTS], BF)
            ybuf = const.tile([128, N_WIN * 32], BF)

            import contextlib
            loop_cm = tc.For_i(0, reps, 1) if reps > 1 else contextlib.nullcontext()
            with loop_cm:
                nc.sync.dma_start(stacked[:32, :], d_xT[:])
                _launch1_body(nc, tc, sbuf, psum, d_slots, d_y, blkT_t, iota_t,
                              wcat_t, stacked, ybuf)

    nc.compile()
    return nc


def _launch1_body(nc, tc, sbuf, psum, d_slots, d_y, blkT_t, iota_t, wcat_t,
                  stacked, ybuf):
    # batch b = (window-group wg, band k): 7 tiles (one per window in the
    # group), all at band k, stream-ordered t' = wg*35 + k*7 + i.
    n_batches = N_WG * TILES_PER_WIN      # 35
    psw = {}
    for b in range(n_batches):
        wg, k = divmod(b, TILES_PER_WIN)
        wk, c0 = BAND_W[k], BAND_C0[k]
        t0 = b * WIN_GROUPS
        bt = sbuf.tile([128, WIN_GROUPS * 256], BF, tag="slots")
        # split the batch load into sub-DMAs on distinct queues: DMA
        # bandwidth scales with concurrently-processing DMA instructions
        bounds = [WIN_GROUPS * s // L1_SPLIT for s in range(L1_SPLIT + 1)]
        for s in range(L1_SPLIT):
            j0, j1 = bounds[s], bounds[s + 1]
            if j0 == j1:
                continue
            _qeng(nc, L1_QS[(b * L1_SPLIT + s) % len(L1_QS)]).dma_start(
                bt[:, j0 * 256:j1 * 256],
                d_slots[:, (t0 + j0) * 256:(t0 + j1) * 256])
        if L1_MODE == "dmaonly":
            continue
        if L1_MODE != "noelem":
            # batched tree-add: 8 slots -> block sums at [:, t, 0:32].
            # stage 1 split Pool/DVE; stages 2-3 + one-hot on DVE
            # (Pool has no is_equal opcode on core v3).
            btv = bt[:].rearrange("b (t sf) -> b t sf", t=WIN_GROUPS)
            if L1_S1 == "dve":
                eng1 = nc.vector
            elif L1_S1 == "pool":
                eng1 = nc.gpsimd
            else:
                eng1 = nc.gpsimd if b % 2 == 0 else nc.vector
            eng1.tensor_tensor(btv[:, :, 0:128], btv[:, :, 0:128],
                               btv[:, :, 128:256], mybir.AluOpType.add)
            nc.vector.tensor_tensor(btv[:, :, 0:64], btv[:, :, 0:64],
                                    btv[:, :, 64:128], mybir.AluOpType.add)
            nc.vector.tensor_tensor(btv[:, :, 0:32], btv[:, :, 0:32],
                                    btv[:, :, 32:64], mybir.AluOpType.add)
            # batched one-hot build, band-narrowed to wk columns (DVE)
            s2 = sbuf.tile([128, WIN_GROUPS * wk], BF, tag="s2",
                           padded_shape=[128, WIN_GROUPS * 26])
            nc.vector.tensor_tensor(
                s2[:].rearrange("p (t f) -> p t f", t=WIN_GROUPS),
                blkT_t[:, t0:t0 + WIN_GROUPS].rearrange(
                    "p (t o) -> p t o", o=1).to_broadcast(
                    [128, WIN_GROUPS, wk]),
                iota_t[:, c0:c0 + wk].rearrange(
                    "p (o f) -> p o f", o=1).to_broadcast(
                    [128, WIN_GROUPS, wk]),
                mybir.AluOpType.is_equal)
        for i in range(WIN_GROUPS):
            w = wg * WIN_GROUPS + i
            q, qr = divmod(i, 3)
            if k == 0 and qr == 0:
                # 3 windows share one PSUM bank via partition thirds
                # (PE PSUM base partition must be 0, 32 or 64)
                psw[(wg, q)] = psum.tile([128, 128], F32, space="PSUM",
                                         tag="pseg", bufs=4,
                                         name=f"pseg{wg}_{q}")
            ps = psw[(wg, q)]
            rhs = (s2[:, i * wk:(i + 1) * wk] if L1_MODE != "noelem"
                   else bt[:, i * 256:i * 256 + wk])
            nc.tensor.matmul(ps[32 * qr:32 * qr + 32, c0:c0 + wk],
                             lhsT=bt[:, i * 256:i * 256 + 32], rhs=rhs,
                             start=True, stop=True)
            if k == TILES_PER_WIN - 1:
                nc.scalar.copy(stacked[32:64, w * 128:(w + 1) * 128],
                               ps[32 * qr:32 * qr + 32, :])
        if k == TILES_PER_WIN - 1 and L1_YTAIL == "group" and \
                L1_MODE in ("full", "noelem"):
            # y for this window group right away: keeps the PE/Act y work
            # spread across the stream instead of a serial iteration tail
            for i in range(WIN_GROUPS):
                u = wg * WIN_GROUPS + i
                py = psum.tile([128, 32], F32, space="PSUM", tag="py")
                nc.tensor.matmul(py[:],
                                 lhsT=stacked[:, u * 128:(u + 1) * 128],
                                 rhs=wcat_t[:], start=True, stop=True)
                nc.scalar.copy(ybuf[:, u * 32:(u + 1) * 32], py[:])

    # y = stacked.T @ wcat, one 128-node chunk at a time, into ybuf
    n_tail = (N_WIN if L1_MODE in ("full", "noelem") and L1_YTAIL != "group"
              else (0 if L1_MODE in ("full", "noelem") else 1))
    for u in range(n_tail):
        py = psum.tile([128, 32], F32, space="PSUM", tag="py")
        nc.tensor.matmul(py[:], lhsT=stacked[:, u * 128:(u + 1) * 128],
                         rhs=wcat_t[:], start=True, stop=True)
        nc.scalar.copy(ybuf[:, u * 32:(u + 1) * 32], py[:])
    # single bulk writeback: d_y[w*128+p, f] = ybuf[p, w*32+f]
    nc.sync.dma_start(d_y[:].rearrange("(w p) f -> p w f", p=128),
                      ybuf[:].rearrange("p (w f) -> p w f", w=N_WIN))


# ---------------------------------------------------------------- launch 2

def _build_launch2(e_pad, reps=1):
    nc = bacc.Bacc("TRN2", target_bir_lowering=False, debug=False,
                   enable_asserts=False, num_devices=N_CORES)
    G = e_pad // 512                      # 128-col groups (512 edges each)
    d_ew = nc.dram_tensor("ew4T", [128, G * 128], EW_DT, kind="ExternalInput")
    d_ys = nc.dram_tensor("ysum", [128, G * 128], BF, kind="ExternalInput")
    d_W4 = nc.dram_tensor("W4", [128, 128], EW_DT, kind="ExternalInput")
    d_out = nc.dram_tensor("out", [128, G * 128], BF, kind="ExternalOutput")

    n_batches = e_pad // EDGE_BATCH
    gpb = EDGE_BATCH // 512               # PE groups per batch
    with tile.TileContext(nc) as tc:
        with tc.tile_pool(name="const", bufs=1) as const, \
             tc.tile_pool(name="sbuf", bufs=L2_BUFS) as sbuf, \
             tc.tile_pool(name="psum", bufs=2, space="PSUM") as psum:
            W4_t = const.tile([128, 128], EW_DT)
            nc.sync.dma_start(W4_t[:], d_W4[:])
            import contextlib
            loop_cm = tc.For_i(0, reps, 1) if reps > 1 else contextlib.nullcontext()
            with loop_cm:
                _launch2_body(nc, tc, sbuf, psum, d_ew, d_ys, d_out, W4_t,
                              n_batches, gpb)

    nc.compile()
    return nc


def _launch2_body(nc, tc, sbuf, psum, d_ew, d_ys, d_out, W4_t, n_batches, gpb):
    C = gpb * 128
    nq = len(L2_QS)
    for b in range(n_batches):
        cols = slice(b * C, (b + 1) * C)
        ewt = sbuf.tile([128, C], EW_DT, tag="ew")
        _qeng(nc, L2_QS[(3 * b) % nq]).dma_start(ewt[:], d_ew[:, cols])
        yst = sbuf.tile([128, C], BF, tag="ys")
        _qeng(nc, L2_QS[(3 * b + 1) % nq]).dma_start(yst[:], d_ys[:, cols])
        if L2_MODE == "dmaonly":
            _qeng(nc, L2_QS[(3 * b + 2) % nq]).dma_start(d_out[:, cols],
                                                         yst[:])
            continue
        pm = psum.tile([128, C], F32, space="PSUM", tag="pm")
        for g in range(gpb):
            gs = slice(g * 128, (g + 1) * 128)
            nc.tensor.matmul(pm[:, gs], lhsT=ewt[:, gs], rhs=W4_t[:],
                             start=True, stop=True)
        outt = sbuf.tile([128, C], BF, tag="out")
        if L2_ADD == "dve" or (L2_ADD == "mix" and b % 2 == 0):
            # DVE adds straight from PSUM (f32 operand, 1x rate)
            nc.vector.tensor_tensor(outt[:], pm[:], yst[:],
                                    mybir.AluOpType.add)
        else:
            # Act drains PSUM to bf16, DVE adds at 2x (Pool can't read PSUM)
            mt = sbuf.tile([128, C], BF, tag="mew")
            nc.scalar.copy(mt[:], pm[:])
            nc.vector.tensor_tensor(outt[:], mt[:], yst[:],
                                    mybir.AluOpType.add)
        _qeng(nc, L2_QS[(3 * b + 2) % nq]).dma_start(d_out[:, cols], outt[:])


# ---------------------------------------------------------------- host side

def _host_prep(x, edge_index, edge_weight):
    """Shard edges by src range, build sorted slot streams + metadata."""
    src = np.asarray(edge_index[0])
    dst = np.asarray(edge_index[1])
    ew = np.asarray(edge_weight, np.float32)
    x = np.asarray(x, np.float32)

    owner = src // NODES_PER_CORE
    prep = {"cores": []}
    q_glob = np.empty(N_NODES, np.int64)

    for c in range(N_CORES):
        eidx = np.nonzero(owner == c)[0]
        s_loc = src[eidx] - c * NODES_PER_CORE
        order = np.argsort(s_loc, kind="stable")
        sid = eidx[order]                     # edge ids sorted by src
        deg = np.bincount(s_loc, minlength=NODES_PER_CORE)
        blocks = (deg + 7) // 8               # 0 for deg-0 nodes

        # --- bin packing: 245 bins (window w, band k); bin (w,k) holds at
        # most BAND_W[k] nodes and 128 blocks.  Snake-deal nodes in
        # descending block order, then swap-repair block overflows.
        caps = np.tile(np.array(BAND_W, np.int64), N_WIN)
        nbins = N_L1_TILES
        norder = np.argsort(-blocks, kind="stable")
        assign = np.empty(NODES_PER_CORE, np.int64)
        load = np.zeros(nbins, np.int64)
        pos, r, fwd = 0, 0, True
        while pos < NODES_PER_CORE:
            elig = np.nonzero(caps > r)[0]
            if not fwd:
                elig = elig[::-1]
            take = min(len(elig), NODES_PER_CORE - pos)
            nodes = norder[pos:pos + take]
            assign[nodes] = elig[:take]
            np.add.at(load, elig[:take], blocks[nodes])
            pos += take
            r += 1
            fwd = not fwd
        for _ in range(2000):
            amax = int(load.argmax())
            if load[amax] <= 128:
                break
            bmin = int(load.argmin())
            nodes_a = np.nonzero(assign == amax)[0]
            nodes_b = np.nonzero(assign == bmin)[0]
            na = nodes_a[np.argmax(blocks[nodes_a])]
            nb = nodes_b[np.argmin(blocks[nodes_b])]
            assign[na], assign[nb] = bmin, amax
            d = blocks[na] - blocks[nb]
            load[amax] -= d
            load[bmin] += d
        assert load.max() <= 128, "bin block overflow; packing failed"

        binw = assign // TILES_PER_WIN
        bink = assign % TILES_PER_WIN
        # index within bin + per-node block offset (in bin-sorted order)
        ordn = np.lexsort((np.arange(NODES_PER_CORE), assign))
        counts = np.bincount(assign, minlength=nbins)
        cum = np.zeros(nbins + 1, np.int64)
        np.cumsum(counts, out=cum[1:])
        idx_in_bin = np.arange(NODES_PER_CORE) - cum[assign[ordn]]
        bb = blocks[ordn]
        cblk = np.cumsum(bb) - bb
        first = np.repeat(cblk[cum[:-1][counts > 0]], counts[counts > 0])
        blk_off_o = cblk - first
        node_slot = np.empty(NODES_PER_CORE, np.int64)
        node_slot[ordn] = np.array(BAND_C0, np.int64)[bink[ordn]] + idx_in_bin
        blk_off = np.empty(NODES_PER_CORE, np.int64)
        blk_off[ordn] = blk_off_o
        node_win = binw

        # tile stream index: t' = (w//7)*35 + k*7 + (w%7)
        tile_of_bin = ((np.arange(nbins) // TILES_PER_WIN) // WIN_GROUPS * 35
                       + (np.arange(nbins) % TILES_PER_WIN) * WIN_GROUPS
                       + (np.arange(nbins) // TILES_PER_WIN) % WIN_GROUPS)
        node_tile = tile_of_bin[assign]

        q_glob[c * NODES_PER_CORE:(c + 1) * NODES_PER_CORE] = \
            c * NODE_SLOTS + node_win * 128 + node_slot

        # block/slot streams (slot row ids into sid, -1 pad)
        edge_start = np.zeros(NODES_PER_CORE + 1, np.int64)
        np.cumsum(deg, out=edge_start[1:])
        slot_idx = np.full(N_L1_TILES * 1024, -1, np.int64)
        blk_rel = np.full(N_L1_TILES * 128, -1, np.int64)
        blk_start = node_tile * 128 + blk_off    # node's first block pos
        tb = int(blocks.sum())
        r_blk = np.arange(tb) - np.repeat(np.cumsum(blocks) - blocks, blocks)
        blk_rel[np.repeat(blk_start, blocks) + r_blk] = \
            np.repeat(node_slot, blocks)
        te = int(deg.sum())
        r_e = np.arange(te) - np.repeat(edge_start[:-1], deg)
        slot_idx[np.repeat(blk_start * 8, deg) + r_e] = \
            np.repeat(edge_start[:-1], deg) + r_e

        # transpose to [128, tiles*8] so device loads are per-partition
        # contiguous: slotsH[p, (t, s, f)] = slot (t*128+p)*8+s
        flat = slot_idx.reshape(N_L1_TILES, 128, 8).transpose(1, 0, 2).reshape(-1)
        ew_slots = np.zeros((flat.size, 32), NPBF)
        valid = flat >= 0
        ew_slots[valid] = ew[sid[flat[valid]]].astype(NPBF)
        ew_slots = ew_slots.reshape(128, N_L1_TILES * 256)

        blkT = blk_rel.reshape(N_L1_TILES, 128).T.astype(NPBF).copy()

        xq = np.zeros((NODE_SLOTS, 32), np.float32)
        xq[node_win * 128 + node_slot] = x[c * NODES_PER_CORE:
                                           (c + 1) * NODES_PER_CORE]

        prep["cores"].append({
            "eidx": eidx, "ew_slots": ew_slots, "blkT": blkT,
            "xT": np.ascontiguousarray(xq.T.astype(NPBF)),
        })

    prep["q_glob"] = q_glob
    prep["src"] = src
    prep["dst"] = dst
    return prep


def _pack_rows(rows, e_pad):
    """[e_pad, 32] -> bf16 [128, (e_pad//4)] with row j, col g*128+b*32+o
    for edge e = g*512 + 4*j + b (matches the launch-2 psum layout)."""
    G = e_pad // 512
    return np.ascontiguousarray(
        rows.reshape(G, 128, 4, 32).transpose(1, 0, 2, 3)
        .reshape(128, G * 128).astype(NPBF))


def _pack_ew4T(rows, e_pad):
    """[e_pad, 32] -> [128, (e_pad//4)] with row b*32+f, col g*128+j
    for edge e = g*512 + 4*j + b (PE lhsT layout, contraction on rows)."""
    G = e_pad // 512
    return np.ascontiguousarray(
        rows.reshape(G, 128, 4, 32).transpose(2, 3, 0, 1)
        .reshape(128, G * 128).astype(NPEW))


def _unpack_out(out_pack, e_pad):
    """inverse of _pack_rows."""
    G = e_pad // 512
    return out_pack.reshape(128, G, 4, 32).transpose(1, 0, 2, 3) \
        .reshape(e_pad, 32)


def _make_iota():
    return np.broadcast_to(np.arange(128, dtype=NPBF), (128, 128)).copy()


def _make_wcat(w_x, w_ew_j):
    return np.concatenate([w_x, w_ew_j], axis=0).astype(NPBF)


def _make_W4(w_ew_i):
    W4 = np.zeros((128, 128), NPEW)
    for cc in range(4):
        W4[cc * 32:(cc + 1) * 32, cc * 32:(cc + 1) * 32] = \
            np.asarray(w_ew_i, np.float32).astype(NPEW)
    return W4


def _in_maps1(prep, w_x, w_ew_j):
    iota = _make_iota()
    wcat = _make_wcat(w_x, w_ew_j)
    return [{"slots": pc["ew_slots"], "blkT": pc["blkT"], "xT": pc["xT"],
             "iota": iota, "wcat": wcat} for pc in prep["cores"]]


def _e_pad(prep):
    e_pad = max(len(pc["eidx"]) for pc in prep["cores"])
    return ((e_pad + EDGE_BATCH - 1) // EDGE_BATCH) * EDGE_BATCH


def _in_maps2(prep, edge_weight, y_q, w_ew_i, e_pad):
    """Host data staging: per-edge gather of y rows (ysum) + layout packs."""
    W4 = _make_W4(w_ew_i)
    ew = np.asarray(edge_weight, np.float32)
    y_f32 = np.asarray(y_q, np.float32)
    qsrc = prep["q_glob"][prep["src"]]
    qdst = prep["q_glob"][prep["dst"]]
    in2 = []
    for pc in prep["cores"]:
        eidx = pc["eidx"]
        n = len(eidx)
        ewb = np.zeros((e_pad, 32), np.float32)
        ewb[:n] = ew[eidx]
        ys = np.zeros((e_pad, 32), np.float32)
        ys[:n] = y_f32[qsrc[eidx]] + y_f32[qdst[eidx]]
        in2.append({"ew4T": _pack_ew4T(ewb, e_pad),
                    "ysum": _pack_rows(ys, e_pad), "W4": W4})
    return in2


def kernel(x, edge_index, edge_weight, w_x, w_ew_i, w_ew_j):
    x = np.asarray(x, np.float32)
    edge_weight = np.asarray(edge_weight, np.float32)
    w_x = np.asarray(w_x, np.float32)
    w_ew_i = np.asarray(w_ew_i, np.float32)
    w_ew_j = np.asarray(w_ew_j, np.float32)
    E = edge_weight.shape[0]

    prep = _host_prep(x, edge_index, edge_weight)

    if "l1" not in _programs:
        _programs["l1"] = _build_launch1()
    nc1 = _programs["l1"]
    res1 = bass_utils.run_bass_kernel_spmd(nc1, _in_maps1(prep, w_x, w_ew_j),
                                           core_ids=list(range(N_CORES)))
    y_q = np.concatenate([res1.results[c]["y"] for c in range(N_CORES)],
                         axis=0)

    e_pad = _e_pad(prep)
    key = ("l2", e_pad)
    if key not in _programs:
        _programs[key] = _build_launch2(e_pad)
    nc2 = _programs[key]
    in2 = _in_maps2(prep, edge_weight, y_q, w_ew_i, e_pad)
    res2 = bass_utils.run_bass_kernel_spmd(nc2, in2,
                                           core_ids=list(range(N_CORES)))

    out = np.empty((E, 32), np.float32)
    for c in range(N_CORES):
        eidx = prep["cores"][c]["eidx"]
        o = _unpack_out(res2.results[c]["out"], e_pad)
        out[eidx] = o[:len(eidx)].astype(np.float32)
    return out
